# revision 24
# baseline (speedup 1.0000x reference)
"""MoE expert-routing kernel for Trainium2 (8 NeuronCores, expert-parallel).

Problem: out[t] = x[t] @ weight[index[t]] + bias[index[t]]
  x: (32768, 512) f32, index: (32768,) int, weight: (8, 512, 512) f32,
  bias: (8, 512) f32.

Strategy (expert-parallel, host-side dispatch):
  Core e owns expert e. The host gathers the tokens routed to expert e
  into a fixed-capacity, transposed buffer xt_e[512, CAP] (padded with
  zeros), and core e computes y_e = x_e @ W_e as a single dense GEMM.
  Results are scattered back to token order on the host, which also
  adds bias[e] (saves the bias DMA + the on-device add). Tokens beyond
  CAP (doesn't happen for the benchmark distribution: observed
  per-expert maxima 4205 vs CAP 4224) fall back to a host matmul, so
  the kernel stays correct for any index distribution.

Numerics: x is shipped in fp8-e3m4 (4 mantissa bits; halves the
  dominant input stream), w in fp16, PSUM accumulates fp32, y returns
  fp16. Verified absmax error 0.078 on scale-5.5 outputs (rel 1.4e-2
  vs the 2e-2 gate).

The kernel is co-designed against three measured hardware budgets:
  1. PE roofline: 33 token-tiles x 4 K-chunk matmuls ([128x128] fp8e3
     stationary x [128x512] fp16 moving) = 132 MMs x 216ns warm = the
     28.5us floor. The steady state measures 216ns/MM exactly (zero
     gaps, LDWEIGHTS fully hidden by the PE reorder window).
  2. HAM clock gate: the PE runs at 1.2 GHz until ~3.4us of sustained
     activity, and any idle gap resets the progress. ~46 tiny
     dependency-free warm-up matmuls fill the preamble/DMA-wait window
     so the clock is at 2.4 GHz before the first real matmul.
  3. Shared DMA drain (~230-250 GB/s/core with all 8 cores active,
     ~2.4us instruction-to-first-byte pipe, ~0.6us HWDGE descriptor
     gen per DMA instruction serialized per ring): the first 512
     tokens are processed as four K-PASSES (k-major) so compute starts
     as soon as w_k0 + the first quarter of x-slab0 land (~10.1us),
     with w_k1..k3 arriving during earlier passes. Ring order pairs
     each w chunk (SP ring) against one x0 k-piece (ACT ring). Steady
     x slabs alternate rings; outputs ride the ACT ring; the final
     tile drains as two halves (DVE + ACT) into DMAs on both rings so
     the ~1.6us completion receipts overlap.

Measured (neuron-profile NTFF, per-core exec): ~45.4-46.5us vs 48.5us
for the PE-stream-only fp16 baseline. Budget: ~7.2us fixed engine
preamble, ~2.9us to first data, ~29.7us supply-paced stream (PE floor
28.5), ~4.9us flush tail + ~1.8us fixed teardown.
"""

import os

import numpy as np

N_EXPERTS = 8
D_IN = 512
D_OUT = 512
N_TOKENS = 32768
CAP = 4224  # per-expert token capacity: 33*128; observed maxima 4205 (int32 seed) / 4166 (x64); host fallback covers overflow
TOK_SLAB = 512
KC = D_IN // 128  # 4 contraction chunks


def _slab_schedule():
    head_sizes = [512]
    tail_sizes = [128, 128]
    sizes = list(head_sizes)
    remaining = CAP - sum(head_sizes) - sum(tail_sizes)
    while remaining > 0:
        sizes.append(min(TOK_SLAB, remaining))
        remaining -= sizes[-1]
    sizes.extend(tail_sizes)
    slabs = []
    t0 = 0
    for ts in sizes:
        slabs.append((t0, ts))
        t0 += ts
    assert t0 == CAP
    return slabs


SLABS = _slab_schedule()
Y_FREE = (CAP // 128) * D_OUT  # packed output free size per partition

# Measured on HW (exec_time / max-abs-err on scale-5.5 outputs):
#   "float32"      ~138us  5.7e-6   exact fp32 (PE 4 cyc/row)
#   "float32r"      ~68us  7.6e-4   fast-fp32 matmul, fp32 out
#   "float32r_o16"  ~53us  2.3e-3   fast-fp32 matmul, fp16 out (DMA-bound)
#   "float16_o16"   ~49us  2.7e-3   fp16 in/out (PE-paced, min DMA) <- default
#   "bfloat16"      ~52us  1.3e-2   bf16 in, fp32 out
MM_DTYPE = os.environ.get("KERNEL_MM_DTYPE", "f8e3x_o16")
# mode -> (x dtype, w dtype, y dtype)
_DT_MAP = {
    "float32": ("float32", "float32", "float32"),
    "float32r": ("float32r", "float32r", "float32"),
    "float32r_o16": ("float32r", "float32r", "float16"),
    "bf16x": ("bfloat16", "float32r", "float32"),
    "bfloat16": ("bfloat16", "bfloat16", "float32"),
    "float16": ("float16", "float16", "float32"),
    "float16_o16": ("float16", "float16", "float16"),
    # x in fp8-e3m4 (4 mantissa bits): halves the dominant x DMA stream.
    # Verified on the benchmark inputs: absmax 0.078 vs 0.111 budget.
    "f8e3x_o16": ("float8e3", "float16", "float16"),
}

_cache = {}


N_WARM = int(os.environ.get("KERNEL_N_WARM", "46"))


def _build(mm_dtype_name):
    import concourse.bacc as bacc
    import concourse.mybir as mybir
    import concourse.tile as tile

    x_dt_name, w_dt_name, y_dt_name = _DT_MAP[mm_dtype_name]
    dt_x = getattr(mybir.dt, x_dt_name)
    dt_w = getattr(mybir.dt, w_dt_name)
    dt_y = getattr(mybir.dt, y_dt_name)
    f32 = mybir.dt.float32

    nc = bacc.Bacc("TRN2", target_bir_lowering=False, debug=False, num_devices=N_EXPERTS)
    # Slab-contiguous packed layouts: one contiguous run per partition
    # per slab DMA (vs 2KB strided chunks for the natural 2D layouts).
    xt = nc.dram_tensor("xt", (128, KC * CAP), dt_x, kind="ExternalInput").ap()
    w = nc.dram_tensor("w", (D_IN, D_OUT), dt_w, kind="ExternalInput").ap()
    y = nc.dram_tensor("y", (128, Y_FREE), dt_y, kind="ExternalOutput").ap()

    with tile.TileContext(nc) as tc:
        with (
            tc.tile_pool(name="wpool", bufs=1) as wpool,
            tc.tile_pool(name="warm", bufs=1) as warm_pool,
            tc.tile_pool(name="xslab", bufs=10) as xpool,
            tc.tile_pool(name="ystage", bufs=6) as ypool,
            tc.tile_pool(name="psum", bufs=6, space="PSUM") as pspool,
            tc.tile_pool(name="warmps", bufs=1, space="PSUM") as warmps_pool,
        ):
            # Slab schedule (module-level, shared with the host packer):
            # small first slabs so matmuls start early, small last slab so
            # the tail flush (DVE + out-DMA after last MM) is short.
            slabs = SLABS

            # Weights: separate tile per k-chunk so the first matmuls only
            # gate on chunk 0 (256KB) instead of the full 1MB.
            w_sbs = [
                wpool.tile([128, D_OUT], dt_w, tag=f"w{k}", name=f"w_sb{k}")
                for k in range(KC)
            ]

            def load_x(slab_i, engine=None):
                t0, ts = slabs[slab_i]
                xs = xpool.tile([128, KC * ts], dt_x, tag="xs")
                (engine or nc.sync).dma_start(xs[:], xt[:, KC * t0 : KC * (t0 + ts)])
                return xs

            # HAM pre-warm: the PE clock sits throttled at 1.2 GHz until
            # ~3.4us of sustained PE activity, and any PE-idle gap resets
            # the busy-window progress. The first real matmul can't start
            # before its DMAs land (~3.2us after the fixed ~7us engine
            # preamble), so fill that window with tiny dependency-free
            # matmuls on a zeroed scratch tile: the HAM un-throttles right
            # around the time the real (gapless) stream begins.
            warm_sb = warm_pool.tile([128, 128], dt_x, tag="warm_sb")
            nc.gpsimd.memset(warm_sb[:], 0.0)
            warm_ps = warmps_pool.tile([64, 64], f32, tag="warm_ps")
            for _ in range(N_WARM):
                nc.tensor.matmul(
                    warm_ps[:], warm_sb[:, 0:64], warm_sb[:, 64:128],
                    start=True, stop=True,
                )

            # DMA choreography. Each HWDGE DMA instruction costs ~0.6us of
            # descriptor-gen on its issuing queue (FIFO!), and data lands
            # ~2.1us + bytes/300GB/s after the instruction retires. A
            # gapless warm MM stream from ~10.6us therefore needs the w
            # chunks and the early x slabs interleaved across BOTH rings,
            # ordered so each lands just before the PE consumes it. All
            # input DMAs are emitted before any output DMA: an out-DMA
            # waiting on its slab's results would head-block the ring FIFO
            # and starve later input loads.
            # Supply order (global drain is ~fair-share across both rings,
            # so ring-position pairs drain together): all four w chunks on
            # SP pair against the ramp slab's four k-chunk pieces on ACT —
            # [w0|x0k0] land first (k0 pass starts ~10.1us), then [w1|x0k1]
            # for the k1 pass, etc. The k-pass pacing (0.86us per pass)
            # rides just behind the ~0.6us/round supply cadence.
            t0_0, ts_0 = slabs[0]
            xs0 = xpool.tile([128, KC * ts_0], dt_x, tag="xs", name="xs_ramp")
            h = D_OUT // 2
            nc.sync.dma_start(w_sbs[0][:, 0:h], w[0:128, 0:h])
            nc.scalar.dma_start(xs0[:, 0:ts_0], xt[:, 0:ts_0])
            nc.sync.dma_start(w_sbs[0][:, h:D_OUT], w[0:128, h:D_OUT])
            nc.scalar.dma_start(xs0[:, ts_0 : 2 * ts_0], xt[:, ts_0 : 2 * ts_0])
            nc.sync.dma_start(w_sbs[1][:], w[128:256, :])
            nc.scalar.dma_start(xs0[:, 2 * ts_0 : 3 * ts_0], xt[:, 2 * ts_0 : 3 * ts_0])
            nc.sync.dma_start(w_sbs[2][:], w[256:384, :])
            nc.scalar.dma_start(xs0[:, 3 * ts_0 : 4 * ts_0], xt[:, 3 * ts_0 : 4 * ts_0])
            nc.sync.dma_start(w_sbs[3][:], w[384:512, :])
            xs_all = [xs0]
            for i in range(1, len(slabs)):
                xs_all.append(load_x(i, nc.sync if i % 2 == 1 else nc.scalar))

            n_slabs = len(slabs)

            def dummy_fill(n):
                for _ in range(n):
                    nc.tensor.matmul(
                        warm_ps[:], warm_sb[:, 0:64], warm_sb[:, 64:128],
                        start=True, stop=True,
                    )

            # k-major ramp: accumulate the first 4 tiles (slabs 0-2) as four
            # k-passes so the PE starts as soon as w0+x0 land (~10.4us) and
            # the later w chunks arrive during earlier passes. Dummy fillers
            # plug the predicted supply stalls so the HAM busy-window isn't
            # reset by PE idle gaps.
            RAMP_SLABS = 1
            ramp_keys = []
            for i in range(RAMP_SLABS):
                t0, ts = slabs[i]
                for a in range(ts // 128):
                    ramp_keys.append((i, a))
            ramp_ps = {}
            for key in ramp_keys:
                ramp_ps[key] = pspool.tile(
                    [128, D_OUT], f32, tag="acc", name=f"acc_r{key[0]}_{key[1]}"
                )
            fill = {}
            for k in range(KC):
                for j, (i, a) in enumerate(ramp_keys):
                    t0, ts = slabs[i]
                    xs_chunk = xs_all[i][:, k * ts + a * 128 : k * ts + (a + 1) * 128]
                    if k == 0:
                        # n-halves: the first MMs gate on only half of w0
                        # (64KB) so compute starts one DMA round earlier.
                        nc.tensor.matmul(
                            ramp_ps[(i, a)][:, 0:h], xs_chunk,
                            w_sbs[0][:, 0:h],
                            start=True, stop=False, skip_group_check=True,
                        )
                        # start=False: the bank-wide has_written clear from
                        # the first half-MM already ran; untouched elements
                        # have has_written=0 so this writes (not adds).
                        nc.tensor.matmul(
                            ramp_ps[(i, a)][:, h:D_OUT], xs_chunk,
                            w_sbs[0][:, h:D_OUT],
                            start=False, stop=False, skip_group_check=True,
                        )
                    else:
                        nc.tensor.matmul(
                            ramp_ps[(i, a)][:],
                            xs_chunk,
                            w_sbs[k][:],
                            start=False, stop=(k == KC - 1),
                            skip_group_check=True,
                        )
                    dummy_fill(fill.get((k, j), 0))
            for i in range(RAMP_SLABS):
                t0, ts = slabs[i]
                nt = ts // 128
                ys = ypool.tile([128, nt * D_OUT], dt_y, tag="ysr", name=f"ys_ramp{i}")
                for a in range(nt):
                    nc.vector.tensor_copy(
                        ys[:, a * D_OUT : (a + 1) * D_OUT], ramp_ps[(i, a)][:]
                    )
                o0 = (t0 // 128) * D_OUT
                nc.scalar.dma_start(y[:, o0 : o0 + nt * D_OUT], ys[:])

            for i, (t0, ts) in enumerate(slabs):
                if i < RAMP_SLABS:
                    continue
                nt = ts // 128
                xs = xs_all[i]
                ys = ypool.tile([128, nt * D_OUT], dt_y, tag="ys")
                last = i == n_slabs - 1
                o_last = (t0 // 128) * D_OUT + (nt - 1) * 0  # last slab is 1 tile
                for a in range(nt):
                    ps = pspool.tile([128, D_OUT], f32, tag="acc")
                    if last and a == nt - 1:
                        # Final tile: contract the two n-halves into SEPARATE
                        # PSUM banks (a shared bank would add a write-after-
                        # read hazard on the first half's drain), flushing
                        # the first half while the second is still on the PE.
                        ps_b = warmps_pool.tile(
                            [128, h], f32, tag="acc_lastB", name="acc_lastB"
                        )
                        for k in range(KC):
                            nc.tensor.matmul(
                                ps[:, 0:h],
                                xs[:, k * ts + a * 128 : k * ts + (a + 1) * 128],
                                w_sbs[k][:, 0:h],
                                start=(k == 0), stop=(k == KC - 1),
                                skip_group_check=True,
                            )
                        nc.vector.tensor_copy(
                            ys[:, a * D_OUT : a * D_OUT + h], ps[:, 0:h]
                        )
                        nc.sync.dma_start(
                            y[:, o_last : o_last + h],
                            ys[:, a * D_OUT : a * D_OUT + h],
                        )
                        for k in range(KC):
                            nc.tensor.matmul(
                                ps_b[:],
                                xs[:, k * ts + a * 128 : k * ts + (a + 1) * 128],
                                w_sbs[k][:, h:D_OUT],
                                start=(k == 0), stop=(k == KC - 1),
                            )
                        nc.scalar.copy(
                            ys[:, a * D_OUT + h : (a + 1) * D_OUT], ps_b[:]
                        )
                        nc.scalar.dma_start(
                            y[:, o_last + h : o_last + D_OUT],
                            ys[:, a * D_OUT + h : (a + 1) * D_OUT],
                        )
                        continue
                    for k in range(KC):
                        nc.tensor.matmul(
                            ps[:],
                            xs[:, k * ts + a * 128 : k * ts + (a + 1) * 128],
                            w_sbs[k][:],
                            start=(k == 0),
                            stop=(k == KC - 1),
                        )
                    if False:
                        # Final tile: drain half on DVE, half on ACT (its
                        # queue is idle; ACT is still busy issuing the prior
                        # slab's out-DMA), so the last PSUM->SBUF hop halves.
                        h = D_OUT // 2
                        nc.vector.tensor_copy(
                            ys[:, a * D_OUT : a * D_OUT + h], ps[:, 0:h]
                        )
                        nc.scalar.copy(
                            ys[:, a * D_OUT + h : (a + 1) * D_OUT], ps[:, h:D_OUT]
                        )
                    else:
                        nc.vector.tensor_copy(
                            ys[:, a * D_OUT : (a + 1) * D_OUT], ps[:]
                        )
                o0 = (t0 // 128) * D_OUT
                if last:
                    pass  # final tile already flushed per-half above
                elif i == n_slabs - 2:
                    # Second-to-last slab drains on the SP ring so the ACT
                    # sequencer is free when the final tile's copy arrives.
                    nc.sync.dma_start(y[:, o0 : o0 + nt * D_OUT], ys[:])
                else:
                    # Output on the ACT HWDGE ring — separate FIFO from inputs.
                    nc.scalar.dma_start(y[:, o0 : o0 + nt * D_OUT], ys[:])
    nc.compile()
    return nc


def _get_nc(mm_dtype_name):
    if mm_dtype_name not in _cache:
        _cache[mm_dtype_name] = _build(mm_dtype_name)
    return _cache[mm_dtype_name]


def kernel(x, index, weight, bias, _trace=False):
    from concourse.bass_utils import run_bass_kernel_spmd

    x = np.ascontiguousarray(np.asarray(x, dtype=np.float32))
    weight = np.ascontiguousarray(np.asarray(weight, dtype=np.float32))
    bias = np.ascontiguousarray(np.asarray(bias, dtype=np.float32))
    idx = np.asarray(index).astype(np.int64, copy=False)

    ids = [np.nonzero(idx == e)[0] for e in range(N_EXPERTS)]

    in_maps = []
    for e in range(N_EXPERTS):
        n_e = min(len(ids[e]), CAP)
        x_e = np.zeros((CAP, D_IN), dtype=np.float32)
        x_e[:n_e] = x[ids[e][:n_e]]
        # Pack slab-major: xt_e[p, KC*t0 + kc*ts + t] = x_e[t0+t, kc*128+p]
        xt_e = np.empty((128, KC * CAP), dtype=np.float32)
        for t0, ts in SLABS:
            blk = x_e[t0 : t0 + ts].reshape(ts, KC, 128)  # [t, kc, p]
            xt_e[:, KC * t0 : KC * (t0 + ts)] = (
                blk.transpose(2, 1, 0).reshape(128, KC * ts)
            )
        in_maps.append({"xt": xt_e, "w": weight[e]})

    x_dt_name, w_dt_name, y_dt_name = _DT_MAP[MM_DTYPE]
    _np_dt = {"bfloat16", "float16", "float8e3"}
    if x_dt_name in _np_dt or w_dt_name in _np_dt:
        import ml_dtypes

        cast = {
            "bfloat16": ml_dtypes.bfloat16,
            "float16": np.float16,
            "float8e3": ml_dtypes.float8_e3m4,
        }
        if x_dt_name in cast:
            in_maps = [
                {**m, "xt": m["xt"].astype(cast[x_dt_name])} for m in in_maps
            ]
        if w_dt_name in cast:
            in_maps = [
                {**m, "w": m["w"].astype(cast[w_dt_name])} for m in in_maps
            ]

    nc = _get_nc(MM_DTYPE)
    res = run_bass_kernel_spmd(
        nc, in_maps, core_ids=list(range(N_EXPERTS)), trace=_trace
    )

    out = np.empty((x.shape[0], D_OUT), dtype=np.float32)
    for e in range(N_EXPERTS):
        n_e = min(len(ids[e]), CAP)
        # Unpack [p, a_global, o] -> token-major [a_global*128+p, o]
        y_pm = res.results[e]["y"].reshape(128, CAP // 128, D_OUT)
        y_e = y_pm.transpose(1, 0, 2).reshape(CAP, D_OUT)
        out[ids[e][:n_e]] = y_e[:n_e].astype(np.float32) + bias[e]
        if len(ids[e]) > CAP:  # capacity overflow: host fallback (correctness net)
            over = ids[e][CAP:]
            out[over] = x[over] @ weight[e] + bias[e]

    if _trace:
        return out, res
    return out



# revision 25
# speedup vs baseline: 1.1824x; 1.1824x over previous
"""MoE expert-routing kernel for Trainium2 (8 NeuronCores, expert-parallel).

Problem: out[t] = x[t] @ weight[index[t]] + bias[index[t]]
  x: (32768, 512) f32, index: (32768,) int, weight: (8, 512, 512) f32,
  bias: (8, 512) f32.

Strategy (expert-parallel, host-side dispatch):
  Core e owns expert e. The host gathers the tokens routed to expert e
  into a fixed-capacity, transposed buffer xt_e[512, CAP] (padded with
  zeros), and core e computes y_e = x_e @ W_e as a single dense GEMM.
  Results are scattered back to token order on the host, which also
  adds bias[e] (saves the bias DMA + the on-device add). Tokens beyond
  CAP (doesn't happen for the benchmark distribution: observed
  per-expert maxima 4205 vs CAP 4224) fall back to a host matmul, so
  the kernel stays correct for any index distribution.

Numerics: x is shipped in fp8-e3m4 (4 mantissa bits; halves the
  dominant input stream), w in fp16, PSUM accumulates fp32, y returns
  fp16. Verified absmax error 0.078 on scale-5.5 outputs (rel 1.4e-2
  vs the 2e-2 gate).

The kernel is co-designed against three measured hardware budgets:
  1. PE roofline: 33 token-tiles x 4 K-chunk matmuls ([128x128] fp8e3
     stationary x [128x512] fp16 moving) = 132 MMs x 216ns warm = the
     28.5us floor. The steady state measures 216ns/MM exactly (zero
     gaps, LDWEIGHTS fully hidden by the PE reorder window).
  2. HAM clock gate: the PE runs at 1.2 GHz until ~3.4us of sustained
     activity, and any idle gap resets the progress. ~46 tiny
     dependency-free warm-up matmuls fill the preamble/DMA-wait window
     so the clock is at 2.4 GHz before the first real matmul.
  3. Shared DMA drain (~230-250 GB/s/core with all 8 cores active,
     ~2.4us instruction-to-first-byte pipe, ~0.6us HWDGE descriptor
     gen per DMA instruction serialized per ring): the first 512
     tokens are processed as four K-PASSES (k-major) so compute starts
     as soon as w_k0 + the first quarter of x-slab0 land (~10.1us),
     with w_k1..k3 arriving during earlier passes. Ring order pairs
     each w chunk (SP ring) against one x0 k-piece (ACT ring). Steady
     x slabs alternate rings; outputs ride the ACT ring; the final
     tile drains as two halves (DVE + ACT) into DMAs on both rings so
     the ~1.6us completion receipts overlap.

Measured (neuron-profile NTFF, per-core exec): ~45.4-46.5us vs 48.5us
for the PE-stream-only fp16 baseline. Budget: ~7.2us fixed engine
preamble, ~2.9us to first data, ~29.7us supply-paced stream (PE floor
28.5), ~4.9us flush tail + ~1.8us fixed teardown.
"""

import os

import numpy as np

N_EXPERTS = 8
D_IN = 512
D_OUT = 512
N_TOKENS = 32768
CAP = 4224  # per-expert token capacity: 33*128; observed maxima 4205 (int32 seed) / 4166 (x64); host fallback covers overflow
TOK_SLAB = 512
KC = D_IN // 128  # 4 contraction chunks


def _slab_schedule():
    head_sizes = [512]
    tail_sizes = [128, 128]
    sizes = list(head_sizes)
    remaining = CAP - sum(head_sizes) - sum(tail_sizes)
    while remaining > 0:
        sizes.append(min(TOK_SLAB, remaining))
        remaining -= sizes[-1]
    sizes.extend(tail_sizes)
    slabs = []
    t0 = 0
    for ts in sizes:
        slabs.append((t0, ts))
        t0 += ts
    assert t0 == CAP
    return slabs


SLABS = _slab_schedule()
Y_FREE = (CAP // 128) * D_OUT  # packed output free size per partition

# Measured on HW (exec_time / max-abs-err on scale-5.5 outputs):
#   "float32"      ~138us  5.7e-6   exact fp32 (PE 4 cyc/row)
#   "float32r"      ~68us  7.6e-4   fast-fp32 matmul, fp32 out
#   "float32r_o16"  ~53us  2.3e-3   fast-fp32 matmul, fp16 out (DMA-bound)
#   "float16_o16"   ~49us  2.7e-3   fp16 in/out (PE-paced, min DMA) <- default
#   "bfloat16"      ~52us  1.3e-2   bf16 in, fp32 out
MM_DTYPE = os.environ.get("KERNEL_MM_DTYPE", "f8e3x_o16")
# mode -> (x dtype, w dtype, y dtype)
_DT_MAP = {
    "float32": ("float32", "float32", "float32"),
    "float32r": ("float32r", "float32r", "float32"),
    "float32r_o16": ("float32r", "float32r", "float16"),
    "bf16x": ("bfloat16", "float32r", "float32"),
    "bfloat16": ("bfloat16", "bfloat16", "float32"),
    "float16": ("float16", "float16", "float32"),
    "float16_o16": ("float16", "float16", "float16"),
    # x in fp8-e3m4 (4 mantissa bits): halves the dominant x DMA stream.
    # Verified on the benchmark inputs: absmax 0.078 vs 0.111 budget.
    "f8e3x_o16": ("float8e3", "float16", "float16"),
}

_cache = {}


N_WARM = int(os.environ.get("KERNEL_N_WARM", "46"))


def _build(mm_dtype_name):
    import concourse.bacc as bacc
    import concourse.mybir as mybir
    import concourse.tile as tile

    x_dt_name, w_dt_name, y_dt_name = _DT_MAP[mm_dtype_name]
    dt_x = getattr(mybir.dt, x_dt_name)
    dt_w = getattr(mybir.dt, w_dt_name)
    dt_y = getattr(mybir.dt, y_dt_name)
    f32 = mybir.dt.float32

    nc = bacc.Bacc("TRN2", target_bir_lowering=False, debug=False, num_devices=N_EXPERTS)
    # Slab-contiguous packed layouts: one contiguous run per partition
    # per slab DMA (vs 2KB strided chunks for the natural 2D layouts).
    xt = nc.dram_tensor("xt", (128, KC * CAP), dt_x, kind="ExternalInput").ap()
    w = nc.dram_tensor("w", (D_IN, D_OUT), dt_w, kind="ExternalInput").ap()
    y = nc.dram_tensor("y", (128, Y_FREE), dt_y, kind="ExternalOutput").ap()

    with tile.TileContext(nc) as tc:
        with (
            tc.tile_pool(name="wpool", bufs=1) as wpool,
            tc.tile_pool(name="warm", bufs=1) as warm_pool,
            tc.tile_pool(name="xslab", bufs=10) as xpool,
            tc.tile_pool(name="ystage", bufs=6) as ypool,
            tc.tile_pool(name="psum", bufs=6, space="PSUM") as pspool,
            tc.tile_pool(name="warmps", bufs=1, space="PSUM") as warmps_pool,
        ):
            # Slab schedule (module-level, shared with the host packer):
            # small first slabs so matmuls start early, small last slab so
            # the tail flush (DVE + out-DMA after last MM) is short.
            slabs = SLABS

            # Weights: separate tile per k-chunk so the first matmuls only
            # gate on chunk 0 (256KB) instead of the full 1MB.
            w_sbs = [
                wpool.tile([128, D_OUT], dt_w, tag=f"w{k}", name=f"w_sb{k}")
                for k in range(KC)
            ]

            def load_x(slab_i, engine=None):
                t0, ts = slabs[slab_i]
                xs = xpool.tile([128, KC * ts], dt_x, tag="xs")
                (engine or nc.sync).dma_start(xs[:], xt[:, KC * t0 : KC * (t0 + ts)])
                return xs

            # HAM pre-warm: the PE clock sits throttled at 1.2 GHz until
            # ~3.4us of sustained PE activity, and any PE-idle gap resets
            # the busy-window progress. The first real matmul can't start
            # before its DMAs land (~3.2us after the fixed ~7us engine
            # preamble), so fill that window with tiny dependency-free
            # matmuls on a zeroed scratch tile: the HAM un-throttles right
            # around the time the real (gapless) stream begins.
            warm_sb = warm_pool.tile([128, 128], dt_x, tag="warm_sb")
            nc.gpsimd.memset(warm_sb[:], 0.0)
            warm_ps = warmps_pool.tile([64, 64], f32, tag="warm_ps")
            for _ in range(N_WARM):
                nc.tensor.matmul(
                    warm_ps[:], warm_sb[:, 0:64], warm_sb[:, 64:128],
                    start=True, stop=True,
                )

            # DMA choreography. Each HWDGE DMA instruction costs ~0.6us of
            # descriptor-gen on its issuing queue (FIFO!), and data lands
            # ~2.1us + bytes/300GB/s after the instruction retires. A
            # gapless warm MM stream from ~10.6us therefore needs the w
            # chunks and the early x slabs interleaved across BOTH rings,
            # ordered so each lands just before the PE consumes it. All
            # input DMAs are emitted before any output DMA: an out-DMA
            # waiting on its slab's results would head-block the ring FIFO
            # and starve later input loads.
            # Supply order (global drain is ~fair-share across both rings,
            # so ring-position pairs drain together): all four w chunks on
            # SP pair against the ramp slab's four k-chunk pieces on ACT —
            # [w0|x0k0] land first (k0 pass starts ~10.1us), then [w1|x0k1]
            # for the k1 pass, etc. The k-pass pacing (0.86us per pass)
            # rides just behind the ~0.6us/round supply cadence.
            t0_0, ts_0 = slabs[0]
            xs0 = xpool.tile([128, KC * ts_0], dt_x, tag="xs", name="xs_ramp")
            h = D_OUT // 2
            nc.sync.dma_start(w_sbs[0][:, 0:h], w[0:128, 0:h])
            nc.scalar.dma_start(xs0[:, 0:ts_0], xt[:, 0:ts_0])
            nc.sync.dma_start(w_sbs[0][:, h:D_OUT], w[0:128, h:D_OUT])
            nc.scalar.dma_start(xs0[:, ts_0 : 2 * ts_0], xt[:, ts_0 : 2 * ts_0])
            nc.sync.dma_start(w_sbs[1][:], w[128:256, :])
            nc.scalar.dma_start(xs0[:, 2 * ts_0 : 3 * ts_0], xt[:, 2 * ts_0 : 3 * ts_0])
            nc.sync.dma_start(w_sbs[2][:], w[256:384, :])
            nc.scalar.dma_start(xs0[:, 3 * ts_0 : 4 * ts_0], xt[:, 3 * ts_0 : 4 * ts_0])
            nc.sync.dma_start(w_sbs[3][:], w[384:512, :])
            xs_all = [xs0]
            for i in range(1, len(slabs)):
                xs_all.append(load_x(i, nc.sync if i % 2 == 1 else nc.scalar))

            n_slabs = len(slabs)

            def dummy_fill(n):
                for _ in range(n):
                    nc.tensor.matmul(
                        warm_ps[:], warm_sb[:, 0:64], warm_sb[:, 64:128],
                        start=True, stop=True,
                    )

            # k-major ramp: accumulate the first 4 tiles (slabs 0-2) as four
            # k-passes so the PE starts as soon as w0+x0 land (~10.4us) and
            # the later w chunks arrive during earlier passes. Dummy fillers
            # plug the predicted supply stalls so the HAM busy-window isn't
            # reset by PE idle gaps.
            RAMP_SLABS = 1
            ramp_keys = []
            for i in range(RAMP_SLABS):
                t0, ts = slabs[i]
                for a in range(ts // 128):
                    ramp_keys.append((i, a))
            ramp_ps = {}
            for key in ramp_keys:
                ramp_ps[key] = pspool.tile(
                    [128, D_OUT], f32, tag="acc", name=f"acc_r{key[0]}_{key[1]}"
                )
            fill = {}
            for k in range(KC):
                for j, (i, a) in enumerate(ramp_keys):
                    t0, ts = slabs[i]
                    xs_chunk = xs_all[i][:, k * ts + a * 128 : k * ts + (a + 1) * 128]
                    if k == 0:
                        # n-halves: the first MMs gate on only half of w0
                        # (64KB) so compute starts one DMA round earlier.
                        nc.tensor.matmul(
                            ramp_ps[(i, a)][:, 0:h], xs_chunk,
                            w_sbs[0][:, 0:h],
                            start=True, stop=False, skip_group_check=True,
                        )
                        # start=False: the bank-wide has_written clear from
                        # the first half-MM already ran; untouched elements
                        # have has_written=0 so this writes (not adds).
                        nc.tensor.matmul(
                            ramp_ps[(i, a)][:, h:D_OUT], xs_chunk,
                            w_sbs[0][:, h:D_OUT],
                            start=False, stop=False, skip_group_check=True,
                        )
                    else:
                        nc.tensor.matmul(
                            ramp_ps[(i, a)][:],
                            xs_chunk,
                            w_sbs[k][:],
                            start=False, stop=(k == KC - 1),
                            skip_group_check=True,
                        )
                    dummy_fill(fill.get((k, j), 0))
            for i in range(RAMP_SLABS):
                t0, ts = slabs[i]
                nt = ts // 128
                ys = ypool.tile([128, nt * D_OUT], dt_y, tag="ysr", name=f"ys_ramp{i}")
                for a in range(nt):
                    nc.vector.tensor_copy(
                        ys[:, a * D_OUT : (a + 1) * D_OUT], ramp_ps[(i, a)][:]
                    )
                o0 = (t0 // 128) * D_OUT
                nc.scalar.dma_start(y[:, o0 : o0 + nt * D_OUT], ys[:])

            for i, (t0, ts) in enumerate(slabs):
                if i < RAMP_SLABS:
                    continue
                nt = ts // 128
                xs = xs_all[i]
                ys = ypool.tile([128, nt * D_OUT], dt_y, tag="ys")
                last = i == n_slabs - 1
                o_last = (t0 // 128) * D_OUT + (nt - 1) * 0  # last slab is 1 tile
                for a in range(nt):
                    ps = pspool.tile([128, D_OUT], f32, tag="acc")
                    if last and a == nt - 1:
                        # Final tile: contract the two n-halves back to back
                        # (same stationary per k -> LDW shared), flush the
                        # first half while the second is still on the PE.
                        for k in range(KC):
                            nc.tensor.matmul(
                                ps[:, 0:h],
                                xs[:, k * ts + a * 128 : k * ts + (a + 1) * 128],
                                w_sbs[k][:, 0:h],
                                start=(k == 0), stop=(k == KC - 1),
                                skip_group_check=True,
                            )
                        nc.vector.tensor_copy(
                            ys[:, a * D_OUT : a * D_OUT + h], ps[:, 0:h]
                        )
                        nc.sync.dma_start(
                            y[:, o_last : o_last + h],
                            ys[:, a * D_OUT : a * D_OUT + h],
                        )
                        for k in range(KC):
                            nc.tensor.matmul(
                                ps[:, h:D_OUT],
                                xs[:, k * ts + a * 128 : k * ts + (a + 1) * 128],
                                w_sbs[k][:, h:D_OUT],
                                start=(k == 0), stop=(k == KC - 1),
                                skip_group_check=True,
                            )
                        nc.scalar.copy(
                            ys[:, a * D_OUT + h : (a + 1) * D_OUT], ps[:, h:D_OUT]
                        )
                        nc.scalar.dma_start(
                            y[:, o_last + h : o_last + D_OUT],
                            ys[:, a * D_OUT + h : (a + 1) * D_OUT],
                        )
                        continue
                    for k in range(KC):
                        nc.tensor.matmul(
                            ps[:],
                            xs[:, k * ts + a * 128 : k * ts + (a + 1) * 128],
                            w_sbs[k][:],
                            start=(k == 0),
                            stop=(k == KC - 1),
                        )
                    if False:
                        # Final tile: drain half on DVE, half on ACT (its
                        # queue is idle; ACT is still busy issuing the prior
                        # slab's out-DMA), so the last PSUM->SBUF hop halves.
                        h = D_OUT // 2
                        nc.vector.tensor_copy(
                            ys[:, a * D_OUT : a * D_OUT + h], ps[:, 0:h]
                        )
                        nc.scalar.copy(
                            ys[:, a * D_OUT + h : (a + 1) * D_OUT], ps[:, h:D_OUT]
                        )
                    else:
                        nc.vector.tensor_copy(
                            ys[:, a * D_OUT : (a + 1) * D_OUT], ps[:]
                        )
                o0 = (t0 // 128) * D_OUT
                if last:
                    pass  # final tile already flushed per-half above
                elif i == n_slabs - 2:
                    # Second-to-last slab drains on the SP ring so the ACT
                    # sequencer is free when the final tile's copy arrives.
                    nc.sync.dma_start(y[:, o0 : o0 + nt * D_OUT], ys[:])
                else:
                    # Output on the ACT HWDGE ring — separate FIFO from inputs.
                    nc.scalar.dma_start(y[:, o0 : o0 + nt * D_OUT], ys[:])
    nc.compile()
    return nc


def _get_nc(mm_dtype_name):
    if mm_dtype_name not in _cache:
        _cache[mm_dtype_name] = _build(mm_dtype_name)
    return _cache[mm_dtype_name]


def kernel(x, index, weight, bias, _trace=False):
    from concourse.bass_utils import run_bass_kernel_spmd

    x = np.ascontiguousarray(np.asarray(x, dtype=np.float32))
    weight = np.ascontiguousarray(np.asarray(weight, dtype=np.float32))
    bias = np.ascontiguousarray(np.asarray(bias, dtype=np.float32))
    idx = np.asarray(index).astype(np.int64, copy=False)

    ids = [np.nonzero(idx == e)[0] for e in range(N_EXPERTS)]

    in_maps = []
    for e in range(N_EXPERTS):
        n_e = min(len(ids[e]), CAP)
        x_e = np.zeros((CAP, D_IN), dtype=np.float32)
        x_e[:n_e] = x[ids[e][:n_e]]
        # Pack slab-major: xt_e[p, KC*t0 + kc*ts + t] = x_e[t0+t, kc*128+p]
        xt_e = np.empty((128, KC * CAP), dtype=np.float32)
        for t0, ts in SLABS:
            blk = x_e[t0 : t0 + ts].reshape(ts, KC, 128)  # [t, kc, p]
            xt_e[:, KC * t0 : KC * (t0 + ts)] = (
                blk.transpose(2, 1, 0).reshape(128, KC * ts)
            )
        in_maps.append({"xt": xt_e, "w": weight[e]})

    x_dt_name, w_dt_name, y_dt_name = _DT_MAP[MM_DTYPE]
    _np_dt = {"bfloat16", "float16", "float8e3"}
    if x_dt_name in _np_dt or w_dt_name in _np_dt:
        import ml_dtypes

        cast = {
            "bfloat16": ml_dtypes.bfloat16,
            "float16": np.float16,
            "float8e3": ml_dtypes.float8_e3m4,
        }
        if x_dt_name in cast:
            in_maps = [
                {**m, "xt": m["xt"].astype(cast[x_dt_name])} for m in in_maps
            ]
        if w_dt_name in cast:
            in_maps = [
                {**m, "w": m["w"].astype(cast[w_dt_name])} for m in in_maps
            ]

    nc = _get_nc(MM_DTYPE)
    res = run_bass_kernel_spmd(
        nc, in_maps, core_ids=list(range(N_EXPERTS)), trace=_trace
    )

    out = np.empty((x.shape[0], D_OUT), dtype=np.float32)
    for e in range(N_EXPERTS):
        n_e = min(len(ids[e]), CAP)
        # Unpack [p, a_global, o] -> token-major [a_global*128+p, o]
        y_pm = res.results[e]["y"].reshape(128, CAP // 128, D_OUT)
        y_e = y_pm.transpose(1, 0, 2).reshape(CAP, D_OUT)
        out[ids[e][:n_e]] = y_e[:n_e].astype(np.float32) + bias[e]
        if len(ids[e]) > CAP:  # capacity overflow: host fallback (correctness net)
            over = ids[e][CAP:]
            out[over] = x[over] @ weight[e] + bias[e]

    if _trace:
        return out, res
    return out

